# revision 11
# baseline (speedup 1.0000x reference)
"""Trainium2 Bass kernel for bidirectional cross-attention (nn_CrossAttention).

Reference computation (per batch b, N=1024 tokens, D=768 dims):
    sim1  = image1 @ image2^T            [N, N]
    out2  = l2norm(softmax(sim1) @ image2) + 2*image2
    sim2  = image2 @ image1^T
    out1  = l2norm(softmax(sim2) @ image1) + 2*image1

Two algebraic simplifications:
  1. l2norm(softmax(S) @ V) == l2norm(exp(S - rowmax) @ V): the softmax
     denominator is a positive per-row scalar cancelled by the L2 norm,
     so the kernel never computes the softmax sum.
  2. sim2 == sim1^T: the kernel computes S = image1 @ image2^T ONCE per
     batch and derives direction 2's scores by PE-transposing the fp16
     SBUF copy of S (8 transposes/tile) instead of a second 49k-cycle
     matmul.

Precision plan (rel-err gate is 2e-2; expected ~1e-3):
  - inputs cast fp32->fp16 during DMA; mm1 in fp16 (1 cycle/row on PE,
    same speed as bf16, ~8x less S error)
  - dir1 softmax: rowmax + exp read S from PSUM fp32 directly
  - dir2 softmax: from the fp16 transpose of S (+-0.03 abs error in the
    exponent -> ~3% on P entries; output is residual-dominated)
  - P matrices stored bf16, transposed on PE, cast to fp8e4 on PSUM
    evacuation
  - both mm2 (P^T.T @ V) run in fp8e4 DoubleRow perf mode: K=256 packed
    per instruction = 2x PE throughput; V in fp8e4 (|V| <= ~5.5 << 448)

Sharding: pure data parallel, B=16 batches -> 2 per core across 8 cores.

PSUM budget (8 banks x 2KB): acc pool 3 x [128,1024]f32 slots shared by
mm1's S tiles and mm2's O tiles (6 banks) + tp16 1 x [128,8,128]f16
(input transposes + S^T groups) + tpP 1 x [128,8,128]bf16 (P transposes).

DIR-phase software pipeline (i = 0..8) per batch:
  dir1(i): P1T transposes + fp8 mm2 -> out2[i]
  ST(i):   S^T transpose group for dir2 row-block i (rowmax DVE, exp ACT)
  dir2(i-1): P2T transposes + fp8 mm2 -> out1[i-1]  (the ST(i-1) ->
             rowmax -> exp chain hides under dir1(i)+ST(i) PE work)
Batch b+1's loads fire at i==0 (ring depth 2 means they only conflict
with batch b-1's readers, which are long done); its input transposes are
injected 2 per iteration. Engine split: ACT = S16 copy, exps, P1T evac,
Square; DVE = rowmaxes, P2T evac, input-transpose evac, residual
doubling, epilogue stt; GPSIMD = load DMA triggers + fp8 natural copies.
"""

import os
import sys

import numpy as np

for _p in ("/opt/trn_rl_repo", "/root/.axon_site/_ro/trn_rl_repo"):
    if os.path.isdir(_p) and _p not in sys.path:
        sys.path.append(_p)

B, N, D = 16, 1024, 768
NCORES = 8
BPC = B // NCORES  # batches per core
P = 128
NT = N // P  # 8 token chunks
DT = D // P  # 6 feature chunks

_PROGRAM_CACHE = {}


def build_program():
    """Build the per-core Bass program (SPMD: identical on all cores)."""
    import concourse.mybir as mybir
    import concourse.tile as tile
    from concourse import bacc
    from concourse.masks import make_identity

    f32 = mybir.dt.float32
    f16 = mybir.dt.float16
    bf16 = mybir.dt.bfloat16
    f8 = mybir.dt.float8e4
    AF = mybir.ActivationFunctionType
    ALU = mybir.AluOpType
    AX = mybir.AxisListType
    DR = mybir.MatmulPerfMode.DoubleRow

    nc = bacc.Bacc(None)
    img_dram = {
        1: nc.declare_dram_parameter("image1", [BPC, N, D], f32, isOutput=False),
        2: nc.declare_dram_parameter("image2", [BPC, N, D], f32, isOutput=False),
    }
    out_dram = {
        1: nc.declare_dram_parameter("out1", [BPC, N, D], f32, isOutput=True),
        2: nc.declare_dram_parameter("out2", [BPC, N, D], f32, isOutput=True),
    }

    with tile.TileContext(nc) as tc:
        with (
            tc.tile_pool(name="const", bufs=1) as const_pool,
            tc.tile_pool(name="nat", bufs=2) as nat_pool,
            tc.tile_pool(name="imgT", bufs=1) as imgT_pool,
            tc.tile_pool(name="s16", bufs=1) as s16_pool,
            tc.tile_pool(name="p1", bufs=NT) as p1_pool,
            tc.tile_pool(name="pw", bufs=2) as pw_pool,
            tc.tile_pool(name="work", bufs=2) as work,
            tc.tile_pool(name="r2", bufs=1) as r2_pool,
            tc.tile_pool(name="sqp", bufs=2) as sq_pool,
            tc.tile_pool(name="outs", bufs=3) as outs,
            tc.tile_pool(name="stats", bufs=8) as stats,
            tc.tile_pool(name="acc", bufs=3, space="PSUM") as acc_pool,
            tc.tile_pool(name="tp16", bufs=1, space="PSUM") as tp16_pool,
            tc.tile_pool(name="tpP", bufs=1, space="PSUM") as tpP_pool,
        ):
            ident16 = const_pool.tile([P, P], f16, tag="id16")
            make_identity(nc, ident16[:])
            identb = const_pool.tile([P, P], bf16, tag="idb")
            make_identity(nc, identb[:])

            nat16 = {}  # (b, im) -> list of 8 fp16 natural chunks [P, D]
            nat8 = {}   # (b, im) -> [P, NT, D] fp8 natural copy (mm2 rhs)
            imgT = {}   # (b, im) -> [P, DT, N] fp16 transposed (mm1 operands)
            s16 = {}    # (b, qi) -> [P, N] fp16 copy of S row-block
            resid2s = {}  # (b, im, kc) -> [P, D] fp16 doubled residual

            def resid_burst(b):
                """Precompute all 16 doubled residual tiles on gpsimd
                (front-loaded so later loads never block the epilogues)."""
                for im, kc in [(2, k) for k in range(NT)] + [
                    (1, k) for k in range(NT)
                ]:
                    r2 = r2_pool.tile(
                        [P, D], f16, tag=f"r2_{im}_{kc}", name="r2"
                    )
                    nc.gpsimd.tensor_scalar_mul(r2[:], nat16[(b, im)][kc][:], 2.0)
                    resid2s[(b, im, kc)] = r2

            def prep_loads(b):
                """image2 via HWDGE fp32 + ACT cast (fast path, feeds mm1
                rhs first); image1 via SWDGE cast-DMA (parallel DMA path).
                fp8 copies: img1 on gpsimd, img2 returned as DVE closures
                to be spread across DIR iterations."""
                for im in (2, 1):
                    nat8[(b, im)] = nat_pool.tile(
                        [P, NT, D], f8, tag=f"nat8_{im}", name=f"nat8_{im}"
                    )
                    chunks = []
                    for kc in range(NT):
                        nb = nat_pool.tile(
                            [P, D], f16, tag=f"nat16_{im}_{kc}", name="nb"
                        )
                        src_ap = img_dram[im][b, kc * P : (kc + 1) * P, :]
                        if im == 2:
                            ldf = work.tile([P, D], f32, tag="ldf")
                            nc.sync.dma_start(ldf[:], src_ap)
                            nc.scalar.activation(nb[:], ldf[:], AF.Copy)
                        else:
                            nc.gpsimd.dma_start(nb[:], src_ap)
                        chunks.append(nb)
                    nat16[(b, im)] = chunks
                for kc in range(NT):
                    nc.gpsimd.tensor_copy(
                        nat8[(b, 1)][:, kc, :], nat16[(b, 1)][kc][:]
                    )

                def make8(kc):
                    def c():
                        nc.vector.tensor_copy(
                            nat8[(b, 2)][:, kc, :], nat16[(b, 2)][kc][:]
                        )
                    return c

                return [make8(kc) for kc in range(NT)]

            def prep_groups(b):
                """Return 16 closures, each PE-transposing one (im, kc) chunk
                into column kc of imgT[im] (6 blocks -> [P, dc, kc*P:...])."""
                tbs = {}
                for im in (2, 1):
                    tbs[im] = imgT_pool.tile(
                        [P, DT, N], f16, tag=f"imgT{im}", name=f"imgT{im}"
                    )
                    imgT[(b, im)] = tbs[im]

                def make(im, kc):
                    def g():
                        nb = nat16[(b, im)][kc]
                        tp = tp16_pool.tile([P, N], f16, tag="tp16")
                        for dc in range(DT):
                            nc.tensor.transpose(
                                tp[:, dc * P : (dc + 1) * P],
                                nb[:, dc * P : (dc + 1) * P],
                                ident16[:],
                            )
                        nc.vector.tensor_copy(
                            tbs[im][:, :, kc * P : (kc + 1) * P],
                            tp[:, : DT * P],
                        )
                    return g

                return [make(im, kc) for im in (2, 1) for kc in range(NT)]

            def mm1(b, qi):
                """S[qi,:] = img1^T.T @ img2^T (fp16), then S16 copy (ACT),
                rowmax (DVE), P1 = exp(S - rowmax) (ACT, fp32 PSUM read)."""
                S = acc_pool.tile([P, N], f32, tag="acc")
                qT = imgT[(b, 1)]
                kT = imgT[(b, 2)]
                for d in range(DT):
                    lhsT = qT[:, d, qi * P : (qi + 1) * P]
                    nc.tensor.matmul(
                        S[:, :512], lhsT, kT[:, d, :512],
                        start=(d == 0), stop=(d == DT - 1),
                    )
                    nc.tensor.matmul(
                        S[:, 512:], lhsT, kT[:, d, 512:],
                        start=(d == 0), stop=(d == DT - 1),
                    )
                sb = s16_pool.tile([P, N], f16, tag=f"s16_{qi}", name="sb")
                s16[(b, qi)] = sb
                nc.scalar.activation(sb[:], S[:], AF.Copy)
                negmax = stats.tile([P, 1], f32, tag="negmax1")
                nc.vector.tensor_reduce(
                    negmax, S[:], axis=AX.X, op=ALU.max, negate=True
                )
                Pw = p1_pool.tile([P, N], bf16, tag="P1")
                nc.scalar.activation(Pw, S[:], AF.Exp, bias=negmax, scale=1.0)
                return Pw

            def mm2(PTs, v8, resid2, out_ap):
                """O = P^T.T @ V in fp8 DoubleRow; l2norm + 2*resid epilogue."""
                Ot = acc_pool.tile([P, N], f32, tag="acc")
                for c in range(3):
                    cs = slice(c * 256, (c + 1) * 256)
                    for g in range(4):
                        nc.tensor.matmul(
                            Ot[:, cs],
                            PTs[:, 2 * g : 2 * g + 2, :],
                            v8[:, 2 * g : 2 * g + 2, cs],
                            start=(g == 0), stop=(g == 3),
                            perf_mode=DR,
                        )
                # epilogue: out = O * rsqrt(sum(O^2)) + 2*resid
                # rsqrt = exp(-0.5*ln(ss)): Ln/Exp/Copy/Square share one ACT
                # table (natural_log_exp_and_others) -> no table reloads
                sq = sq_pool.tile([P, D], bf16, tag="sq")
                ss = stats.tile([P, 1], f32, tag="ss")
                nc.scalar.activation(sq, Ot[:, :D], AF.Square, accum_out=ss)
                lss = stats.tile([P, 1], f32, tag="lss")
                nc.scalar.activation(lss, ss, AF.Ln)
                inv = stats.tile([P, 1], f32, tag="inv")
                nc.scalar.activation(inv, lss, AF.Exp, scale=-0.5)
                T3 = outs.tile([P, D], f32, tag="T3")
                nc.vector.scalar_tensor_tensor(
                    out=T3, in0=Ot[:, :D], scalar=inv, in1=resid2[:],
                    op0=ALU.mult, op1=ALU.add,
                )
                nc.sync.dma_start(out_ap, T3[:])

            def dir1_iter(b, qi, Pw):
                """P1T transposes (bf16), fp8 evac on ACT, mm2 -> out2[qi]."""
                tp = tpP_pool.tile([P, N], bf16, tag="tpP")
                for kc in range(NT):
                    nc.tensor.transpose(
                        tp[:, kc * P : (kc + 1) * P],
                        Pw[:, kc * P : (kc + 1) * P], identb[:]
                    )
                PTs = pw_pool.tile([P, NT, P], f8, tag="P1Ts")
                nc.scalar.activation(PTs[:], tp[:], AF.Copy)
                mm2(
                    PTs, nat8[(b, 2)], resid2s[(b, 2, qi)],
                    out_dram[2][b, qi * P : (qi + 1) * P, :],
                )

            def st_group(b, mi):
                """Transpose S16 column-block mi -> ST psum [P, NT, P] fp16,
                then rowmax (DVE) + exp (ACT) -> P2 bf16."""
                tp = tp16_pool.tile([P, N], f16, tag="tp16")
                for qi in range(NT):
                    nc.tensor.transpose(
                        tp[:, qi * P : (qi + 1) * P],
                        s16[(b, qi)][:, mi * P : (mi + 1) * P],
                        ident16[:],
                    )
                negmax = stats.tile([P, 1], f32, tag="negmax2")
                nc.vector.tensor_reduce(
                    negmax, tp[:], axis=AX.X, op=ALU.max, negate=True
                )
                P2 = pw_pool.tile([P, N], bf16, tag="P2")
                nc.scalar.activation(P2, tp[:], AF.Exp, bias=negmax, scale=1.0)
                return P2

            def dir2_iter(b, mi, P2):
                """P2T transposes (fp8), evac on DVE, mm2 -> out1[mi]."""
                tp = tpP_pool.tile([P, N], bf16, tag="tpP")
                for kc in range(NT):
                    nc.tensor.transpose(
                        tp[:, kc * P : (kc + 1) * P],
                        P2[:, kc * P : (kc + 1) * P], identb[:]
                    )
                PTs = pw_pool.tile([P, NT, P], f8, tag="P2Ts")
                nc.vector.tensor_copy(PTs[:], tp[:])
                mm2(
                    PTs, nat8[(b, 1)], resid2s[(b, 1, mi)],
                    out_dram[1][b, mi * P : (mi + 1) * P, :],
                )

            # ---- schedule ----
            n8c = prep_loads(0)
            for c in n8c:
                c()
            for g in prep_groups(0):
                g()
            for b in range(BPC):
                P1s = {qi: mm1(b, qi) for qi in range(NT)}
                pending_groups = []
                pending_n8 = []
                P2_prev = None
                resid_burst(b)
                for i in range(NT + 1):
                    if i < NT:
                        dir1_iter(b, i, P1s.pop(i))
                        P2_cur = st_group(b, i)
                    else:
                        P2_cur = None
                    if i == 0 and b + 1 < BPC:
                        pending_n8 = prep_loads(b + 1)
                        pending_groups = prep_groups(b + 1)
                    if P2_prev is not None:
                        dir2_iter(b, i - 1, P2_prev)
                    P2_prev = P2_cur
                    if pending_n8 and i >= 1:
                        for c in pending_n8[:2]:
                            c()
                        pending_n8 = pending_n8[2:]
                    if pending_groups and i >= 1:
                        for g in pending_groups[:2]:
                            g()
                        pending_groups = pending_groups[2:]

    return nc


def _get_program():
    if "nc" not in _PROGRAM_CACHE:
        nc = build_program()
        if not nc.is_finalized():
            nc.finalize()
        _PROGRAM_CACHE["nc"] = nc
    return _PROGRAM_CACHE["nc"]


def kernel(image1: np.ndarray, image2: np.ndarray):
    from concourse.bass_utils import run_bass_kernel_spmd

    image1 = np.ascontiguousarray(image1, dtype=np.float32)
    image2 = np.ascontiguousarray(image2, dtype=np.float32)
    assert image1.shape == (B, N, D) and image2.shape == (B, N, D)

    nc = _get_program()
    core_ids = list(range(NCORES))
    in_maps = [
        {
            "image1": image1[c * BPC : (c + 1) * BPC],
            "image2": image2[c * BPC : (c + 1) * BPC],
        }
        for c in core_ids
    ]
    res = run_bass_kernel_spmd(nc, in_maps, core_ids)
    out1 = np.concatenate([res.results[c]["out1"] for c in core_ids], axis=0)
    out2 = np.concatenate([res.results[c]["out2"] for c in core_ids], axis=0)
    return out1, out2


# revision 12
# speedup vs baseline: 1.4075x; 1.4075x over previous
"""Trainium2 Bass kernel for bidirectional cross-attention (nn_CrossAttention).

Reference computation (per batch b, N=1024 tokens, D=768 dims):
    sim1  = image1 @ image2^T            [N, N]
    out2  = l2norm(softmax(sim1) @ image2) + 2*image2
    sim2  = image2 @ image1^T
    out1  = l2norm(softmax(sim2) @ image1) + 2*image1

Two algebraic simplifications:
  1. l2norm(softmax(S) @ V) == l2norm(exp(S - rowmax) @ V): the softmax
     denominator is a positive per-row scalar cancelled by the L2 norm,
     so the kernel never computes the softmax sum.
  2. sim2 == sim1^T: the kernel computes S = image1 @ image2^T ONCE per
     batch and derives direction 2's scores by PE-transposing the fp16
     SBUF copy of S (8 transposes/tile) instead of a second 49k-cycle
     matmul.

Precision plan (rel-err gate is 2e-2; expected ~1e-3):
  - inputs cast fp32->fp16 during DMA; mm1 in fp16 (1 cycle/row on PE,
    same speed as bf16, ~8x less S error)
  - dir1 softmax: rowmax + exp read S from PSUM fp32 directly
  - dir2 softmax: from the fp16 transpose of S (+-0.03 abs error in the
    exponent -> ~3% on P entries; output is residual-dominated)
  - P matrices stored bf16, transposed on PE, cast to fp8e4 on PSUM
    evacuation
  - both mm2 (P^T.T @ V) run in fp8e4 DoubleRow perf mode: K=256 packed
    per instruction = 2x PE throughput; V in fp8e4 (|V| <= ~5.5 << 448)

Sharding: pure data parallel, B=16 batches -> 2 per core across 8 cores.

PSUM budget (8 banks x 2KB): acc pool 3 x [128,1024]f32 slots shared by
mm1's S tiles and mm2's O tiles (6 banks) + tp16 1 x [128,8,128]f16
(input transposes + S^T groups) + tpP 1 x [128,8,128]bf16 (P transposes).

DIR-phase software pipeline (i = 0..8) per batch:
  dir1(i): P1T transposes + fp8 mm2 -> out2[i]
  ST(i):   S^T transpose group for dir2 row-block i (rowmax DVE, exp ACT)
  dir2(i-1): P2T transposes + fp8 mm2 -> out1[i-1]  (the ST(i-1) ->
             rowmax -> exp chain hides under dir1(i)+ST(i) PE work)
Batch b+1's loads fire at i==0 (ring depth 2 means they only conflict
with batch b-1's readers, which are long done); its input transposes are
injected 2 per iteration. Engine split: ACT = S16 copy, exps, P1T evac,
Square; DVE = rowmaxes, P2T evac, input-transpose evac, residual
doubling, epilogue stt; GPSIMD = load DMA triggers + fp8 natural copies.
"""

import os
import sys

import numpy as np

for _p in ("/opt/trn_rl_repo", "/root/.axon_site/_ro/trn_rl_repo"):
    if os.path.isdir(_p) and _p not in sys.path:
        sys.path.append(_p)

B, N, D = 16, 1024, 768
NCORES = 8
BPC = B // NCORES  # batches per core
P = 128
NT = N // P  # 8 token chunks
DT = D // P  # 6 feature chunks

_PROGRAM_CACHE = {}


def build_program():
    """Build the per-core Bass program (SPMD: identical on all cores)."""
    import concourse.mybir as mybir
    import concourse.tile as tile
    from concourse import bacc
    from concourse.masks import make_identity

    f32 = mybir.dt.float32
    f16 = mybir.dt.float16
    bf16 = mybir.dt.bfloat16
    f8 = mybir.dt.float8e4
    AF = mybir.ActivationFunctionType
    ALU = mybir.AluOpType
    AX = mybir.AxisListType
    DR = mybir.MatmulPerfMode.DoubleRow

    nc = bacc.Bacc(None)
    img_dram = {
        1: nc.declare_dram_parameter("image1", [BPC, N, D], f32, isOutput=False),
        2: nc.declare_dram_parameter("image2", [BPC, N, D], f32, isOutput=False),
    }
    out_dram = {
        1: nc.declare_dram_parameter("out1", [BPC, N, D], f32, isOutput=True),
        2: nc.declare_dram_parameter("out2", [BPC, N, D], f32, isOutput=True),
    }

    with tile.TileContext(nc) as tc:
        with (
            tc.tile_pool(name="const", bufs=1) as const_pool,
            tc.tile_pool(name="nat", bufs=2) as nat_pool,
            tc.tile_pool(name="imgT", bufs=1) as imgT_pool,
            tc.tile_pool(name="s16", bufs=1) as s16_pool,
            tc.tile_pool(name="p1", bufs=NT) as p1_pool,
            tc.tile_pool(name="pw", bufs=2) as pw_pool,
            tc.tile_pool(name="work", bufs=2) as work,
            tc.tile_pool(name="sqp", bufs=2) as sq_pool,
            tc.tile_pool(name="outs", bufs=3) as outs,
            tc.tile_pool(name="stats", bufs=8) as stats,
            tc.tile_pool(name="acc", bufs=3, space="PSUM") as acc_pool,
            tc.tile_pool(name="tp16", bufs=1, space="PSUM") as tp16_pool,
            tc.tile_pool(name="tpP", bufs=1, space="PSUM") as tpP_pool,
        ):
            ident16 = const_pool.tile([P, P], f16, tag="id16")
            make_identity(nc, ident16[:])
            identb = const_pool.tile([P, P], bf16, tag="idb")
            make_identity(nc, identb[:])

            nat16 = {}  # (b, im) -> list of 8 fp16 natural chunks [P, D]
            nat8 = {}   # (b, im) -> [P, NT, D] fp8 natural copy (mm2 rhs)
            imgT = {}   # (b, im) -> [P, DT, N] fp16 transposed (mm1 operands)
            s16 = {}    # (b, qi) -> [P, N] fp16 copy of S row-block
            def prep_loads(b):
                """Both images via HWDGE fp32 + ACT cast with scale=2.0:
                nat16 = 2*image (fp16).  The doubled tensor IS the epilogue
                residual; the 4x on S is undone by exp's 0.25 scale and the
                2x on V cancels in the l2 norm.  Returns fp8-copy closures
                (nat8_1 on ACT, nat8_2 on DVE) to spread across iterations."""
                for im in (2, 1):
                    nat8[(b, im)] = nat_pool.tile(
                        [P, NT, D], f8, tag=f"nat8_{im}", name=f"nat8_{im}"
                    )
                    chunks = []
                    for kc in range(NT):
                        nb = nat_pool.tile(
                            [P, D], f16, tag=f"nat16_{im}_{kc}", name="nb"
                        )
                        ldf = work.tile([P, D], f32, tag="ldf")
                        nc.sync.dma_start(
                            ldf[:], img_dram[im][b, kc * P : (kc + 1) * P, :]
                        )
                        nc.scalar.activation(nb[:], ldf[:], AF.Copy, scale=2.0)
                        chunks.append(nb)
                    nat16[(b, im)] = chunks

                def make8(im, kc):
                    def c():
                        eng = nc.scalar if im == 1 else nc.vector
                        if im == 1:
                            nc.scalar.activation(
                                nat8[(b, im)][:, kc, :],
                                nat16[(b, im)][kc][:], AF.Copy,
                            )
                        else:
                            nc.vector.tensor_copy(
                                nat8[(b, im)][:, kc, :], nat16[(b, im)][kc][:]
                            )
                    return c

                return [make8(im, kc) for im in (2, 1) for kc in range(NT)]

            def prep_groups(b):
                """Return 16 closures, each PE-transposing one (im, kc) chunk
                into column kc of imgT[im] (6 blocks -> [P, dc, kc*P:...])."""
                tbs = {}
                for im in (2, 1):
                    tbs[im] = imgT_pool.tile(
                        [P, DT, N], f16, tag=f"imgT{im}", name=f"imgT{im}"
                    )
                    imgT[(b, im)] = tbs[im]

                def make(im, kc):
                    def g():
                        nb = nat16[(b, im)][kc]
                        tp = tp16_pool.tile([P, N], f16, tag="tp16")
                        for dc in range(DT):
                            nc.tensor.transpose(
                                tp[:, dc * P : (dc + 1) * P],
                                nb[:, dc * P : (dc + 1) * P],
                                ident16[:],
                            )
                        nc.vector.tensor_copy(
                            tbs[im][:, :, kc * P : (kc + 1) * P],
                            tp[:, : DT * P],
                        )
                    return g

                return [make(im, kc) for im in (2, 1) for kc in range(NT)]

            def mm1(b, qi):
                """S[qi,:] = img1^T.T @ img2^T (fp16), then S16 copy (ACT),
                rowmax (DVE), P1 = exp(S - rowmax) (ACT, fp32 PSUM read)."""
                S = acc_pool.tile([P, N], f32, tag="acc")
                qT = imgT[(b, 1)]
                kT = imgT[(b, 2)]
                for d in range(DT):
                    lhsT = qT[:, d, qi * P : (qi + 1) * P]
                    nc.tensor.matmul(
                        S[:, :512], lhsT, kT[:, d, :512],
                        start=(d == 0), stop=(d == DT - 1),
                    )
                    nc.tensor.matmul(
                        S[:, 512:], lhsT, kT[:, d, 512:],
                        start=(d == 0), stop=(d == DT - 1),
                    )
                sb = s16_pool.tile([P, N], f16, tag=f"s16_{qi}", name="sb")
                s16[(b, qi)] = sb
                nc.scalar.activation(sb[:], S[:], AF.Copy)
                negmax = stats.tile([P, 1], f32, tag="negmax1")
                nc.vector.tensor_reduce(
                    negmax, S[:], axis=AX.X, op=ALU.max, negate=True
                )
                negmax4 = stats.tile([P, 1], f32, tag="negmax14")
                nc.vector.tensor_scalar_mul(negmax4, negmax, 0.25)
                Pw = p1_pool.tile([P, N], bf16, tag="P1")
                nc.scalar.activation(Pw, S[:], AF.Exp, bias=negmax4, scale=0.25)
                return Pw

            def mm2(PTs, v8, resid2x, out_ap):
                """O = P^T.T @ V in fp8 DoubleRow; l2norm + 2*resid epilogue."""
                Ot = acc_pool.tile([P, N], f32, tag="acc")
                for c in range(3):
                    cs = slice(c * 256, (c + 1) * 256)
                    for g in range(4):
                        nc.tensor.matmul(
                            Ot[:, cs],
                            PTs[:, 2 * g : 2 * g + 2, :],
                            v8[:, 2 * g : 2 * g + 2, cs],
                            start=(g == 0), stop=(g == 3),
                            perf_mode=DR,
                        )
                # epilogue: out = O * rsqrt(sum(O^2)) + 2*resid
                # rsqrt = exp(-0.5*ln(ss)): Ln/Exp/Copy/Square share one ACT
                # table (natural_log_exp_and_others) -> no table reloads
                sq = sq_pool.tile([P, D], bf16, tag="sq")
                ss = stats.tile([P, 1], f32, tag="ss")
                nc.scalar.activation(sq, Ot[:, :D], AF.Square, accum_out=ss)
                lss = stats.tile([P, 1], f32, tag="lss")
                nc.scalar.activation(lss, ss, AF.Ln)
                inv = stats.tile([P, 1], f32, tag="inv")
                nc.scalar.activation(inv, lss, AF.Exp, scale=-0.5)
                T3 = outs.tile([P, D], f32, tag="T3")
                nc.vector.scalar_tensor_tensor(
                    out=T3, in0=Ot[:, :D], scalar=inv, in1=resid2x[:],
                    op0=ALU.mult, op1=ALU.add,
                )
                nc.sync.dma_start(out_ap, T3[:])

            def dir1_iter(b, qi, Pw):
                """P1T transposes (bf16), fp8 evac on ACT, mm2 -> out2[qi]."""
                tp = tpP_pool.tile([P, N], bf16, tag="tpP")
                for kc in range(NT):
                    nc.tensor.transpose(
                        tp[:, kc * P : (kc + 1) * P],
                        Pw[:, kc * P : (kc + 1) * P], identb[:]
                    )
                PTs = pw_pool.tile([P, NT, P], f8, tag="P1Ts")
                nc.scalar.activation(PTs[:], tp[:], AF.Copy)
                mm2(
                    PTs, nat8[(b, 2)], nat16[(b, 2)][qi],
                    out_dram[2][b, qi * P : (qi + 1) * P, :],
                )

            def st_group(b, mi):
                """Transpose S16 column-block mi -> ST psum [P, NT, P] fp16,
                then rowmax (DVE) + exp (ACT) -> P2 bf16."""
                tp = tp16_pool.tile([P, N], f16, tag="tp16")
                for qi in range(NT):
                    nc.tensor.transpose(
                        tp[:, qi * P : (qi + 1) * P],
                        s16[(b, qi)][:, mi * P : (mi + 1) * P],
                        ident16[:],
                    )
                negmax = stats.tile([P, 1], f32, tag="negmax2")
                nc.vector.tensor_reduce(
                    negmax, tp[:], axis=AX.X, op=ALU.max, negate=True
                )
                negmax4 = stats.tile([P, 1], f32, tag="negmax24")
                nc.vector.tensor_scalar_mul(negmax4, negmax, 0.25)
                P2 = pw_pool.tile([P, N], bf16, tag="P2")
                nc.scalar.activation(P2, tp[:], AF.Exp, bias=negmax4, scale=0.25)
                return P2

            def dir2_iter(b, mi, P2):
                """P2T transposes (fp8), evac on DVE, mm2 -> out1[mi]."""
                tp = tpP_pool.tile([P, N], bf16, tag="tpP")
                for kc in range(NT):
                    nc.tensor.transpose(
                        tp[:, kc * P : (kc + 1) * P],
                        P2[:, kc * P : (kc + 1) * P], identb[:]
                    )
                PTs = pw_pool.tile([P, NT, P], f8, tag="P2Ts")
                nc.vector.tensor_copy(PTs[:], tp[:])
                mm2(
                    PTs, nat8[(b, 1)], nat16[(b, 1)][mi],
                    out_dram[1][b, mi * P : (mi + 1) * P, :],
                )

            # ---- schedule ----
            n8c = prep_loads(0)
            for c in n8c:
                c()
            for g in prep_groups(0):
                g()
            for b in range(BPC):
                P1s = {qi: mm1(b, qi) for qi in range(NT)}
                pending_groups = []
                pending_n8 = []
                P2_prev = None
                for i in range(NT + 1):
                    if i < NT:
                        dir1_iter(b, i, P1s.pop(i))
                        P2_cur = st_group(b, i)
                    else:
                        P2_cur = None
                    if i == 0 and b + 1 < BPC:
                        pending_n8 = prep_loads(b + 1)
                        pending_groups = prep_groups(b + 1)
                    if P2_prev is not None:
                        dir2_iter(b, i - 1, P2_prev)
                    P2_prev = P2_cur
                    if pending_n8 and i >= 1:
                        for c in pending_n8[:2]:
                            c()
                        pending_n8 = pending_n8[2:]
                    if pending_groups and i >= 1:
                        for g in pending_groups[:2]:
                            g()
                        pending_groups = pending_groups[2:]

    return nc


def _get_program():
    if "nc" not in _PROGRAM_CACHE:
        nc = build_program()
        if not nc.is_finalized():
            nc.finalize()
        _PROGRAM_CACHE["nc"] = nc
    return _PROGRAM_CACHE["nc"]


def kernel(image1: np.ndarray, image2: np.ndarray):
    from concourse.bass_utils import run_bass_kernel_spmd

    image1 = np.ascontiguousarray(image1, dtype=np.float32)
    image2 = np.ascontiguousarray(image2, dtype=np.float32)
    assert image1.shape == (B, N, D) and image2.shape == (B, N, D)

    nc = _get_program()
    core_ids = list(range(NCORES))
    in_maps = [
        {
            "image1": image1[c * BPC : (c + 1) * BPC],
            "image2": image2[c * BPC : (c + 1) * BPC],
        }
        for c in core_ids
    ]
    res = run_bass_kernel_spmd(nc, in_maps, core_ids)
    out1 = np.concatenate([res.results[c]["out1"] for c in core_ids], axis=0)
    out2 = np.concatenate([res.results[c]["out2"] for c in core_ids], axis=0)
    return out1, out2


# revision 13
# speedup vs baseline: 1.6865x; 1.1982x over previous
"""Trainium2 Bass kernel for bidirectional cross-attention (nn_CrossAttention).

Reference computation (per batch b, N=1024 tokens, D=768 dims):
    sim1  = image1 @ image2^T            [N, N]
    out2  = l2norm(softmax(sim1) @ image2) + 2*image2
    sim2  = image2 @ image1^T
    out1  = l2norm(softmax(sim2) @ image1) + 2*image1

Two algebraic simplifications:
  1. l2norm(softmax(S) @ V) == l2norm(exp(S - rowmax) @ V): the softmax
     denominator is a positive per-row scalar cancelled by the L2 norm,
     so the kernel never computes the softmax sum.
  2. sim2 == sim1^T: the kernel computes S = image1 @ image2^T ONCE per
     batch and derives direction 2's scores by PE-transposing the fp16
     SBUF copy of S (8 transposes/tile) instead of a second 49k-cycle
     matmul.

Precision plan (rel-err gate is 2e-2; expected ~1e-3):
  - inputs cast fp32->fp16 during DMA; mm1 in fp16 (1 cycle/row on PE,
    same speed as bf16, ~8x less S error)
  - dir1 softmax: rowmax + exp read S from PSUM fp32 directly
  - dir2 softmax: from the fp16 transpose of S (+-0.03 abs error in the
    exponent -> ~3% on P entries; output is residual-dominated)
  - P matrices stored bf16, transposed on PE, cast to fp8e4 on PSUM
    evacuation
  - both mm2 (P^T.T @ V) run in fp8e4 DoubleRow perf mode: K=256 packed
    per instruction = 2x PE throughput; V in fp8e4 (|V| <= ~5.5 << 448)

Sharding: pure data parallel, B=16 batches -> 2 per core across 8 cores.

PSUM budget (8 banks x 2KB): acc pool 3 x [128,1024]f32 slots shared by
mm1's S tiles and mm2's O tiles (6 banks) + tp16 1 x [128,8,128]f16
(input transposes + S^T groups) + tpP 1 x [128,8,128]bf16 (P transposes).

DIR-phase software pipeline (i = 0..8) per batch:
  dir1(i): P1T transposes + fp8 mm2 -> out2[i]
  ST(i):   S^T transpose group for dir2 row-block i (rowmax DVE, exp ACT)
  dir2(i-1): P2T transposes + fp8 mm2 -> out1[i-1]  (the ST(i-1) ->
             rowmax -> exp chain hides under dir1(i)+ST(i) PE work)
Batch b+1's loads fire at i==0 (ring depth 2 means they only conflict
with batch b-1's readers, which are long done); its input transposes are
injected 2 per iteration. Engine split: ACT = S16 copy, exps, P1T evac,
Square; DVE = rowmaxes, P2T evac, input-transpose evac, residual
doubling, epilogue stt; GPSIMD = load DMA triggers + fp8 natural copies.
"""

import os
import sys

import numpy as np

for _p in ("/opt/trn_rl_repo", "/root/.axon_site/_ro/trn_rl_repo"):
    if os.path.isdir(_p) and _p not in sys.path:
        sys.path.append(_p)

B, N, D = 16, 1024, 768
NCORES = 8
BPC = B // NCORES  # batches per core
P = 128
NT = N // P  # 8 token chunks
DT = D // P  # 6 feature chunks

_PROGRAM_CACHE = {}


def build_program():
    """Build the per-core Bass program (SPMD: identical on all cores)."""
    import concourse.mybir as mybir
    import concourse.tile as tile
    from concourse import bacc
    from concourse.masks import make_identity

    f32 = mybir.dt.float32
    f16 = mybir.dt.float16
    bf16 = mybir.dt.bfloat16
    f8 = mybir.dt.float8e4
    AF = mybir.ActivationFunctionType
    ALU = mybir.AluOpType
    AX = mybir.AxisListType
    DR = mybir.MatmulPerfMode.DoubleRow

    nc = bacc.Bacc(None)
    img_dram = {
        1: nc.declare_dram_parameter("image1", [BPC, N, D], f32, isOutput=False),
        2: nc.declare_dram_parameter("image2", [BPC, N, D], f32, isOutput=False),
    }
    out_dram = {
        1: nc.declare_dram_parameter("out1", [BPC, N, D], f32, isOutput=True),
        2: nc.declare_dram_parameter("out2", [BPC, N, D], f32, isOutput=True),
    }

    with tile.TileContext(nc) as tc:
        with (
            tc.tile_pool(name="const", bufs=1) as const_pool,
            tc.tile_pool(name="nat", bufs=2) as nat_pool,
            tc.tile_pool(name="imgT", bufs=1) as imgT_pool,
            tc.tile_pool(name="s16", bufs=1) as s16_pool,
            tc.tile_pool(name="p1", bufs=NT) as p1_pool,
            tc.tile_pool(name="pw", bufs=2) as pw_pool,
            tc.tile_pool(name="work", bufs=2) as work,
            tc.tile_pool(name="sqp", bufs=2) as sq_pool,
            tc.tile_pool(name="t5", bufs=16) as t5_pool,
            tc.tile_pool(name="outs", bufs=3) as outs,
            tc.tile_pool(name="stats", bufs=8) as stats,
            tc.tile_pool(name="acc", bufs=3, space="PSUM") as acc_pool,
            tc.tile_pool(name="tp16", bufs=1, space="PSUM") as tp16_pool,
            tc.tile_pool(name="tpP", bufs=1, space="PSUM") as tpP_pool,
        ):
            ident16 = const_pool.tile([P, P], f16, tag="id16")
            make_identity(nc, ident16[:])
            identb = const_pool.tile([P, P], bf16, tag="idb")
            make_identity(nc, identb[:])

            nat16 = {}  # (b, im) -> list of 8 fp16 natural chunks [P, D]
            nat8 = {}   # (b, im) -> [P, NT, D] fp8 natural copy (mm2 rhs)
            imgT = {}   # (b, im) -> [P, DT, N] fp16 transposed (mm1 operands)
            s16 = {}    # (b, qi) -> [P, N] fp16 copy of S row-block
            def prep_loads(b):
                """Both images via HWDGE fp32 + ACT cast with scale=2.0:
                nat16 = 2*image (fp16).  The doubled tensor IS the epilogue
                residual; the 4x on S is undone by exp's 0.25 scale and the
                2x on V cancels in the l2 norm.  Returns fp8-copy closures
                (nat8_1 on ACT, nat8_2 on DVE) to spread across iterations."""
                for im in (2, 1):
                    nat8[(b, im)] = nat_pool.tile(
                        [P, NT, D], f8, tag=f"nat8_{im}", name=f"nat8_{im}"
                    )
                    chunks = []
                    for kc in range(NT):
                        nb = nat_pool.tile(
                            [P, D], f16, tag=f"nat16_{im}_{kc}", name="nb"
                        )
                        ldf = work.tile([P, D], f32, tag="ldf")
                        nc.sync.dma_start(
                            ldf[:], img_dram[im][b, kc * P : (kc + 1) * P, :]
                        )
                        nc.scalar.activation(nb[:], ldf[:], AF.Copy, scale=2.0)
                        chunks.append(nb)
                    nat16[(b, im)] = chunks

                def make8(im, kc):
                    def c():
                        if im == 1:
                            nc.scalar.activation(
                                nat8[(b, im)][:, kc, :],
                                nat16[(b, im)][kc][:], AF.Copy,
                            )
                        else:
                            nc.vector.tensor_copy(
                                nat8[(b, im)][:, kc, :], nat16[(b, im)][kc][:]
                            )
                    return c

                return [make8(im, kc) for im in (2, 1) for kc in range(NT)]

            def prep_groups(b):
                """Return 16 closures, each PE-transposing one (im, kc) chunk
                into column kc of imgT[im] (6 blocks -> [P, dc, kc*P:...])."""
                tbs = {}
                for im in (2, 1):
                    tbs[im] = imgT_pool.tile(
                        [P, DT, N], f16, tag=f"imgT{im}", name=f"imgT{im}"
                    )
                    imgT[(b, im)] = tbs[im]

                def make(im, kc):
                    def g():
                        nb = nat16[(b, im)][kc]
                        tp = tp16_pool.tile([P, N], f16, tag="tp16")
                        for dc in range(DT):
                            nc.tensor.transpose(
                                tp[:, dc * P : (dc + 1) * P],
                                nb[:, dc * P : (dc + 1) * P],
                                ident16[:],
                            )
                        nc.vector.tensor_copy(
                            tbs[im][:, :, kc * P : (kc + 1) * P],
                            tp[:, : DT * P],
                        )
                    return g

                return [make(im, kc) for im in (2, 1) for kc in range(NT)]

            def mm1(b, qi):
                """S[qi,:] = img1^T.T @ img2^T (fp16), then S16 copy (ACT),
                rowmax (DVE), P1 = exp(S - rowmax) (ACT, fp32 PSUM read)."""
                S = acc_pool.tile([P, N], f32, tag="acc")
                qT = imgT[(b, 1)]
                kT = imgT[(b, 2)]
                for d in range(DT):
                    lhsT = qT[:, d, qi * P : (qi + 1) * P]
                    nc.tensor.matmul(
                        S[:, :512], lhsT, kT[:, d, :512],
                        start=(d == 0), stop=(d == DT - 1),
                    )
                    nc.tensor.matmul(
                        S[:, 512:], lhsT, kT[:, d, 512:],
                        start=(d == 0), stop=(d == DT - 1),
                    )
                sb = s16_pool.tile([P, N], f16, tag=f"s16_{qi}", name="sb")
                s16[(b, qi)] = sb
                nc.scalar.activation(sb[:], S[:], AF.Copy)
                negmax = stats.tile([P, 1], f32, tag="negmax1")
                nc.vector.tensor_reduce(
                    negmax, S[:], axis=AX.X, op=ALU.max, negate=True
                )
                negmax4 = stats.tile([P, 1], f32, tag="negmax14")
                nc.vector.tensor_scalar_mul(negmax4, negmax, 0.25)
                Pw = p1_pool.tile([P, N], bf16, tag="P1")
                nc.scalar.activation(Pw, S[:], AF.Exp, bias=negmax4, scale=0.25)
                return Pw

            def mm2(PTs, v8, resid2x, out_ap, ss16, idx, epi_q):
                """O = P^T.T @ V in fp8 DoubleRow; Square+accum -> ss16[:,idx];
                O evacuated to SBUF bf16.  Scale + stt + store deferred to the
                per-batch epilogue burst (one Ln+Exp pair per batch keeps the
                ACT table fixed on natural_log_exp_and_others)."""
                Ot = acc_pool.tile([P, N], f32, tag="acc")
                for c in range(3):
                    cs = slice(c * 256, (c + 1) * 256)
                    for g in range(4):
                        nc.tensor.matmul(
                            Ot[:, cs],
                            PTs[:, 2 * g : 2 * g + 2, :],
                            v8[:, 2 * g : 2 * g + 2, cs],
                            start=(g == 0), stop=(g == 3),
                            perf_mode=DR,
                        )
                sq = sq_pool.tile([P, D], bf16, tag="sq")
                nc.scalar.activation(
                    sq, Ot[:, :D], AF.Square, accum_out=ss16[:, idx : idx + 1]
                )
                T5 = t5_pool.tile([P, D], bf16, tag="T5")
                nc.vector.tensor_copy(T5[:], Ot[:, :D])
                epi_q.append((T5, idx, resid2x, out_ap))

            def epilogue_burst(ss16, epi_q):
                """inv = rsqrt(ss) batched: exp(-0.5*ln(ss)) on [P,16], then
                16 stt + store ops."""
                lss = stats.tile([P, 16], f32, tag="lss16")
                nc.scalar.activation(lss, ss16[:], AF.Ln)
                inv16 = stats.tile([P, 16], f32, tag="inv16")
                nc.scalar.activation(inv16, lss, AF.Exp, scale=-0.5)
                for T5, idx, resid, out_ap in epi_q:
                    T3 = outs.tile([P, D], f32, tag="T3")
                    nc.vector.scalar_tensor_tensor(
                        out=T3, in0=T5[:], scalar=inv16[:, idx : idx + 1],
                        in1=resid[:], op0=ALU.mult, op1=ALU.add,
                    )
                    nc.sync.dma_start(out_ap, T3[:])

            def dir1_iter(b, qi, Pw, ss16, epi_q):
                """P1T transposes (bf16), fp8 evac on ACT, mm2 -> out2[qi]."""
                tp = tpP_pool.tile([P, N], bf16, tag="tpP")
                for kc in range(NT):
                    nc.tensor.transpose(
                        tp[:, kc * P : (kc + 1) * P],
                        Pw[:, kc * P : (kc + 1) * P], identb[:]
                    )
                PTs = pw_pool.tile([P, NT, P], f8, tag="P1Ts")
                nc.scalar.activation(PTs[:], tp[:], AF.Copy)
                mm2(
                    PTs, nat8[(b, 2)], nat16[(b, 2)][qi],
                    out_dram[2][b, qi * P : (qi + 1) * P, :],
                    ss16, 2 * qi, epi_q,
                )

            def st_group(b, mi):
                """Transpose S16 column-block mi -> ST psum [P, NT, P] fp16,
                then rowmax (DVE) + exp (ACT) -> P2 bf16."""
                tp = tp16_pool.tile([P, N], f16, tag="tp16")
                for qi in range(NT):
                    nc.tensor.transpose(
                        tp[:, qi * P : (qi + 1) * P],
                        s16[(b, qi)][:, mi * P : (mi + 1) * P],
                        ident16[:],
                    )
                negmax = stats.tile([P, 1], f32, tag="negmax2")
                nc.vector.tensor_reduce(
                    negmax, tp[:], axis=AX.X, op=ALU.max, negate=True
                )
                negmax4 = stats.tile([P, 1], f32, tag="negmax24")
                nc.vector.tensor_scalar_mul(negmax4, negmax, 0.25)
                P2 = pw_pool.tile([P, N], bf16, tag="P2")
                nc.scalar.activation(P2, tp[:], AF.Exp, bias=negmax4, scale=0.25)
                return P2

            def dir2_iter(b, mi, P2, ss16, epi_q):
                """P2T transposes (fp8), evac on DVE, mm2 -> out1[mi]."""
                tp = tpP_pool.tile([P, N], bf16, tag="tpP")
                for kc in range(NT):
                    nc.tensor.transpose(
                        tp[:, kc * P : (kc + 1) * P],
                        P2[:, kc * P : (kc + 1) * P], identb[:]
                    )
                PTs = pw_pool.tile([P, NT, P], f8, tag="P2Ts")
                nc.vector.tensor_copy(PTs[:], tp[:])
                mm2(
                    PTs, nat8[(b, 1)], nat16[(b, 1)][mi],
                    out_dram[1][b, mi * P : (mi + 1) * P, :],
                    ss16, 2 * mi + 1, epi_q,
                )

            # ---- schedule ----
            n8c = prep_loads(0)
            for c in n8c:
                c()
            for g in prep_groups(0):
                g()
            for b in range(BPC):
                P1s = {qi: mm1(b, qi) for qi in range(NT)}
                ss16 = stats.tile([P, 16], f32, tag="ss16", name="ss16")
                epi_q = []
                pending_groups = []
                pending_n8 = []
                P2_prev = None
                for i in range(NT + 1):
                    if i < NT:
                        dir1_iter(b, i, P1s.pop(i), ss16, epi_q)
                        P2_cur = st_group(b, i)
                    else:
                        P2_cur = None
                    if i == 0 and b + 1 < BPC:
                        pending_n8 = prep_loads(b + 1)
                        pending_groups = prep_groups(b + 1)
                    if P2_prev is not None:
                        dir2_iter(b, i - 1, P2_prev, ss16, epi_q)
                    P2_prev = P2_cur
                    if pending_n8 and i >= 1:
                        for c in pending_n8[:2]:
                            c()
                        pending_n8 = pending_n8[2:]
                    if pending_groups and i >= 1:
                        for g in pending_groups[:2]:
                            g()
                        pending_groups = pending_groups[2:]
                epilogue_burst(ss16, epi_q)

    return nc


def _get_program():
    if "nc" not in _PROGRAM_CACHE:
        nc = build_program()
        if not nc.is_finalized():
            nc.finalize()
        _PROGRAM_CACHE["nc"] = nc
    return _PROGRAM_CACHE["nc"]


def kernel(image1: np.ndarray, image2: np.ndarray):
    from concourse.bass_utils import run_bass_kernel_spmd

    image1 = np.ascontiguousarray(image1, dtype=np.float32)
    image2 = np.ascontiguousarray(image2, dtype=np.float32)
    assert image1.shape == (B, N, D) and image2.shape == (B, N, D)

    nc = _get_program()
    core_ids = list(range(NCORES))
    in_maps = [
        {
            "image1": image1[c * BPC : (c + 1) * BPC],
            "image2": image2[c * BPC : (c + 1) * BPC],
        }
        for c in core_ids
    ]
    res = run_bass_kernel_spmd(nc, in_maps, core_ids)
    out1 = np.concatenate([res.results[c]["out1"] for c in core_ids], axis=0)
    out2 = np.concatenate([res.results[c]["out2"] for c in core_ids], axis=0)
    return out1, out2
